# revision 1
# baseline (speedup 1.0000x reference)
"""TextLSTM kernel for 8 Trainium2 NeuronCores.

Data-parallel over batch: each of the 8 cores runs the full model on a
512-row batch shard.

Per-core pipeline (feature-major LSTM):
  1. Embedding gather: indirect-DMA 2560 rows of Emb (t-major token order)
     into SBUF batch-major, PE-transpose 128x128 blocks into feature-major
     xT[t] tiles (cast bf16).
  2. LSTM recurrence, 5 steps: gates[4H, 512b] = WT.T @ [h; x_t] computed as
     128x128 (bf16 weight stationary) x [128, 512] (bf16 h/x moving) matmuls
     accumulated in fp32 PSUM; sigmoid/tanh on ScalarE (gate bias folded in);
     cell math fp32 on VectorE; h stored bf16 (double-buffered), c fp32.
  3. Projection: out[512b, 32000v] = h.T @ WoutT streamed per 512-col vocab
     tile (bf16 weights, fp32 PSUM/output).

Weights are pre-transposed/tiled/cast on the host; biases are all zero per
the problem spec (gate biases are still applied on-device via the activation
bias port; bout is added on host only if nonzero).
"""

import os
import sys

import numpy as np
import ml_dtypes

for _p in ("/opt/trn_rl_repo", "/root/.axon_site/_ro/trn_rl_repo"):
    if os.path.isdir(_p) and _p not in sys.path:
        sys.path.append(_p)

from concourse import bacc, mybir
import concourse.tile as tile
from concourse.bass import IndirectOffsetOnAxis
from concourse.bass_utils import run_bass_kernel_spmd
from concourse.masks import make_identity

P = 128
B, T, E, H, V = 4096, 5, 512, 1024, 32000
NCORES = 8
BS = B // NCORES          # 512 batch rows per core
NTOK = BS * T             # 2560 gathered tokens per core
NG = NTOK // P            # 20 gather tiles of 128 tokens
KH = H // P               # 8 k-tiles over h
KE = E // P               # 4 k-tiles over x
KHX = KH + KE             # 12 k-tiles over [h; x]
NJ = H // P               # 8 hidden-dim tiles
VN = 512                  # vocab tile width
VT = (V + VN - 1) // VN   # 63 vocab tiles (last one 256 wide)
VPAD = VT * VN            # 32256
NBT = BS // P             # 4 batch tiles

F32 = mybir.dt.float32
BF16 = mybir.dt.bfloat16
I32 = mybir.dt.int32
AF = mybir.ActivationFunctionType

_BF = ml_dtypes.bfloat16

_CACHE = {}
LAST_RESULTS = None


def _build():
    nc = bacc.Bacc("TRN2", target_bir_lowering=False, debug=False,
                   num_devices=NCORES)

    idx_d = nc.dram_tensor("idx", [P, NG], I32, kind="ExternalInput")
    emb_d = nc.dram_tensor("emb", [V, E], BF16, kind="ExternalInput")
    wt_d = nc.dram_tensor("wt", [P, KHX, 4 * H], BF16, kind="ExternalInput")
    bias_d = nc.dram_tensor("bias", [P, 4 * H // P], F32, kind="ExternalInput")
    wo_d = nc.dram_tensor("wo", [VT, P, KH * VN], BF16, kind="ExternalInput")
    out_d = nc.dram_tensor("out", [BS, V], F32, kind="ExternalOutput")

    with tile.TileContext(nc) as tc:
        with (
            tc.tile_pool(name="const", bufs=1) as cpool,
            tc.tile_pool(name="gather", bufs=6) as gpool,
            tc.tile_pool(name="work", bufs=2) as wpool,
            tc.tile_pool(name="woutp", bufs=3) as wopool,
            tc.tile_pool(name="outp", bufs=4) as opool,
            tc.tile_pool(name="psum", bufs=8, space="PSUM") as pspool,
        ):
            ident = cpool.tile([P, P], BF16, tag="ident")
            make_identity(nc, ident[:])

            # persistent SBUF state
            wt_sb = cpool.tile([P, KHX, 4 * H], BF16, tag="wt")
            bias_sb = cpool.tile([P, 4 * H // P], F32, tag="bias")
            h_sb = cpool.tile([P, 2, KH, BS], BF16, tag="h")
            c_sb = cpool.tile([P, NJ, BS], F32, tag="c")
            xt_sb = cpool.tile([P, T, KE, BS], BF16, tag="xt")
            idx_sb = cpool.tile([P, NG], I32, tag="idx")

            nc.sync.dma_start(out=idx_sb[:], in_=idx_d.ap())
            nc.sync.dma_start(out=bias_sb[:], in_=bias_d.ap())
            # x-part weights (k 8..11) first: they gate the t=0 matmuls,
            # the h-part loads overlap with t=0 compute.
            for kt in list(range(KH, KHX)) + list(range(KH)):
                nc.sync.dma_start(out=wt_sb[:, kt, :], in_=wt_d.ap()[:, kt, :])

            # all embedding gathers issued upfront; they pipeline on the
            # dynamic DMA queue well ahead of the recurrence consuming them.
            xgs = []
            for g in range(NG):
                xg = gpool.tile([P, E], BF16, tag="xg")
                nc.gpsimd.indirect_dma_start(
                    out=xg[:],
                    out_offset=None,
                    in_=emb_d.ap(),
                    in_offset=IndirectOffsetOnAxis(ap=idx_sb[:, g:g + 1], axis=0),
                )
                xgs.append(xg)

            # PE-transpose one step's gather tiles into feature-major
            def emit_transposes(tt):
                for bb in range(NBT):
                    xg = xgs[tt * NBT + bb]
                    for e in range(KE):
                        ps_tr = pspool.tile([P, P], BF16, tag="ps",
                                            name="ps_tr")
                        nc.tensor.transpose(
                            ps_tr[:], xg[:, e * P:(e + 1) * P], ident[:])
                        nc.vector.tensor_copy(
                            out=xt_sb[:, tt, e, bb * P:(bb + 1) * P],
                            in_=ps_tr[:])

            # ---- LSTM recurrence ----
            emit_transposes(0)
            for t in range(T):
                rbuf, wbuf = t % 2, (t + 1) % 2
                # x-part k-tiles first: their rhs (xt) is ready immediately,
                # so PE enters the step while the tail of the previous
                # step's h writes is still in flight.
                ks = (list(range(KH, KHX)) + list(range(KH)) if t > 0
                      else list(range(KH, KHX)))

                for j in range(NJ):
                    # next step's transposes go mid-stream, where PSUM slots
                    # are freshly recycled — not at the step boundary where
                    # they'd contend with the previous step's gate drains
                    if j == 1 and t + 1 < T:
                        emit_transposes(t + 1)
                    gate_ps = []
                    for gi in range(4):
                        ps = pspool.tile([P, VN], F32, tag="ps")
                        col = gi * H + j * P
                        for n, k in enumerate(ks):
                            rhs = (h_sb[:, rbuf, k, :] if k < KH
                                   else xt_sb[:, t, k - KH, :])
                            nc.tensor.matmul(
                                ps[:],
                                lhsT=wt_sb[:, k, col:col + P],
                                rhs=rhs,
                                start=(n == 0),
                                stop=(n == len(ks) - 1),
                            )
                        gate_ps.append(ps)

                    bcol = lambda gi: bias_sb[:, gi * NJ + j:gi * NJ + j + 1]
                    f_sb = wpool.tile([P, BS], F32, tag="f")
                    i_sb = wpool.tile([P, BS], F32, tag="i")
                    g_sb = wpool.tile([P, BS], F32, tag="g")
                    o_sb = wpool.tile([P, BS], F32, tag="o")
                    nc.scalar.activation(f_sb[:], gate_ps[0][:], AF.Sigmoid,
                                         bias=bcol(0))
                    nc.scalar.activation(i_sb[:], gate_ps[1][:], AF.Sigmoid,
                                         bias=bcol(1))
                    nc.scalar.activation(g_sb[:], gate_ps[2][:], AF.Tanh,
                                         bias=bcol(2))
                    nc.scalar.activation(o_sb[:], gate_ps[3][:], AF.Sigmoid,
                                         bias=bcol(3))

                    if t == 0:
                        nc.vector.tensor_mul(out=c_sb[:, j, :], in0=i_sb[:],
                                             in1=g_sb[:])
                    else:
                        # in-place: c *= f; g_sb = i*g; c += g_sb
                        nc.vector.tensor_mul(out=c_sb[:, j, :], in0=f_sb[:],
                                             in1=c_sb[:, j, :])
                        nc.vector.tensor_mul(out=g_sb[:], in0=i_sb[:],
                                             in1=g_sb[:])
                        nc.vector.tensor_add(out=c_sb[:, j, :],
                                             in0=c_sb[:, j, :], in1=g_sb[:])
                    th = wpool.tile([P, BS], F32, tag="th")
                    nc.scalar.activation(th[:], c_sb[:, j, :], AF.Tanh)
                    nc.vector.tensor_mul(out=h_sb[:, wbuf, j, :], in0=o_sb[:],
                                         in1=th[:])

            # ---- output projection ----
            hbuf = T % 2
            QW = KH * VN // 4  # wout tile loaded in 4 quarters for overlap
            for vt in range(VT):
                vn = min(VN, V - vt * VN)
                wo_sb = wopool.tile([P, KH * VN], BF16, tag="wo")
                for q in range(4):
                    nc.sync.dma_start(out=wo_sb[:, q * QW:(q + 1) * QW],
                                      in_=wo_d.ap()[vt][:, q * QW:(q + 1) * QW])
                for bt in range(NBT):
                    ps = pspool.tile([P, VN], F32, tag="ps")
                    for k in range(KH):
                        nc.tensor.matmul(
                            ps[:, :vn],
                            lhsT=h_sb[:, hbuf, k, bt * P:(bt + 1) * P],
                            rhs=wo_sb[:, k * VN:k * VN + vn],
                            start=(k == 0),
                            stop=(k == KH - 1),
                        )
                    ot = opool.tile([P, VN], F32, tag="ot")
                    nc.vector.tensor_copy(out=ot[:, :vn], in_=ps[:, :vn])
                    # logit writes go out on the ACT HWDGE queue so they
                    # don't contend with the wout reads on the sync queue
                    nc.scalar.dma_start(
                        out=out_d.ap()[bt * P:(bt + 1) * P,
                                       vt * VN:vt * VN + vn],
                        in_=ot[:, :vn])

    nc.compile()
    return nc


def get_nc():
    if "nc" not in _CACHE:
        _CACHE["nc"] = _build()
    return _CACHE["nc"]


def _prep_shared(Emb, WF, WI, WC, WO, bF, bI, bC, bO, Wout):
    emb = np.ascontiguousarray(np.asarray(Emb, dtype=np.float32)).astype(_BF)

    WT = np.concatenate([np.asarray(WF), np.asarray(WI), np.asarray(WC),
                         np.asarray(WO)], 0).astype(np.float32).T  # [1536, 4096]
    wt = np.ascontiguousarray(
        WT.reshape(KHX, P, 4 * H).transpose(1, 0, 2)).astype(_BF)  # [128,12,4096]

    b_all = np.concatenate([np.asarray(bF), np.asarray(bI), np.asarray(bC),
                            np.asarray(bO)], 0).astype(np.float32)  # [4096]
    bias = np.ascontiguousarray(b_all.reshape(4 * H // P, P).T)  # [128, 32]

    Wout = np.asarray(Wout, dtype=np.float32)
    wpad = np.zeros((VPAD, H), np.float32)
    wpad[:V] = Wout
    wo = np.ascontiguousarray(
        wpad.reshape(VT, VN, KH, P).transpose(0, 3, 2, 1).reshape(VT, P, KH * VN)
    ).astype(_BF)  # [63, 128, 4096]
    return emb, wt, bias, wo


def kernel(X, Emb, WF, bF, WI, bI, WC, bC, WO, bO, Wout, bout):
    global LAST_RESULTS
    nc = get_nc()

    emb, wt, bias, wo = _prep_shared(Emb, WF, WI, WC, WO, bF, bI, bC, bO, Wout)
    X = np.asarray(X).astype(np.int32)  # [4096, 5]

    in_maps = []
    for c in range(NCORES):
        xs = X[c * BS:(c + 1) * BS]                       # [512, 5]
        idx = np.ascontiguousarray(
            xs.T.reshape(NG, P).T).astype(np.int32)       # [128, 20] t-major
        in_maps.append({"idx": idx, "emb": emb, "wt": wt,
                        "bias": bias, "wo": wo})

    res = run_bass_kernel_spmd(nc, in_maps, core_ids=list(range(NCORES)))
    LAST_RESULTS = res

    out = np.concatenate([res.results[c]["out"] for c in range(NCORES)], 0)
    bout = np.asarray(bout, dtype=np.float32)
    if np.any(bout):
        out = out + bout[None, :]
    return out



# revision 5
# speedup vs baseline: 1.1901x; 1.1901x over previous
"""TextLSTM kernel for 8 Trainium2 NeuronCores.

Data-parallel over batch: each of the 8 cores runs the full model on a
512-row batch shard.

Key structure (v3):
  1. Host folds Emb @ Wx.T + b into a [32000, 4096] bf16 table, so the
     x-contribution of every gate pre-activation is a single indirect-DMA
     gather (t-major, 8KB rows) -- no x matmuls and no t=0 matmuls at all
     (h0 = 0).
  2. Gates are computed batch-major: pre[b, gate] = h_fm.T @ Wh + xc.
     The h-part runs as fp8(e4m3) DoubleRow matmuls (K=256 per instr,
     0.5 cyc/row): lhsT = h feature-major fp8 [128, 2, 128], moving = Wh
     fp8 [128, 2, 512], fp32 PSUM. The gather tile is added on VectorE
     ((psum * 2^-21) + xc) and gates activate on ScalarE.
  3. h transposes to feature-major via SBUF->SBUF DMA-transpose (bf16,
     on the ACT HWDGE queue), then one quantize-to-fp8 op per batch tile.
     Cell state c stays fp32 batch-major in SBUF.
  4. Projection: out[512b, 32000v] = h5.T @ WoutT per 512-col vocab tile
     (bf16 weights streamed on the sync queue, fp32 PSUM, outputs on the
     ACT queue). h5 stays bf16 (fp8 would break the 2e-2 error budget;
     measured ~3.7e-2 vs 8.5e-3 for fp8-LSTM-only).
"""

import os
import sys

import numpy as np
import ml_dtypes

for _p in ("/opt/trn_rl_repo", "/root/.axon_site/_ro/trn_rl_repo"):
    if os.path.isdir(_p) and _p not in sys.path:
        sys.path.append(_p)

from concourse import bacc, mybir
import concourse.tile as tile
from concourse.bass import IndirectOffsetOnAxis
from concourse.bass_utils import run_bass_kernel_spmd

P = 128
B, T, E, H, V = 4096, 5, 512, 1024, 32000
NCORES = 8
BS = B // NCORES          # 512 batch rows per core
NBT = BS // P             # 4 batch tiles
NG = NBT * T              # 20 gather tiles of 128 tokens
G4 = 4 * H                # 4096 gate pre-activations per token
KH = H // P               # 8 k-tiles over h
NQ = KH // 2              # 4 DoubleRow k-pairs
VN = 512                  # vocab tile width
VT = (V + VN - 1) // VN   # 63 vocab tiles (last one 256 wide)
VPAD = VT * VN            # 32256

SC_H = 8192.0             # h -> fp8 scale (2^13)
SC_W = 256.0              # Wh -> fp8 scale (2^8)
DESCALE = 1.0 / (SC_H * SC_W)

F32 = mybir.dt.float32
BF16 = mybir.dt.bfloat16
FP8 = mybir.dt.float8e4
I32 = mybir.dt.int32
AF = mybir.ActivationFunctionType
ALU = mybir.AluOpType
DR = mybir.MatmulPerfMode.DoubleRow

_BF = ml_dtypes.bfloat16
_F8 = ml_dtypes.float8_e4m3fn

_CACHE = {}
LAST_RESULTS = None


def _build():
    nc = bacc.Bacc("TRN2", target_bir_lowering=False, debug=False,
                   num_devices=NCORES)

    idx_d = nc.dram_tensor("idx", [P, NG], I32, kind="ExternalInput")
    xt_d = nc.dram_tensor("xt", [V, G4], BF16, kind="ExternalInput")
    wh_d = nc.dram_tensor("wh", [P, NQ, 2, G4], FP8, kind="ExternalInput")
    wo_d = nc.dram_tensor("wo", [VT, P, KH * VN], BF16, kind="ExternalInput")
    out_d = nc.dram_tensor("out", [BS, V], F32, kind="ExternalOutput")

    with tile.TileContext(nc) as tc:
        with (
            tc.tile_pool(name="const", bufs=1) as cpool,
            tc.tile_pool(name="gather", bufs=6) as gpool,
            tc.tile_pool(name="hstate", bufs=1) as hpool,
            tc.tile_pool(name="hbmp", bufs=3) as hbmpool,
            tc.tile_pool(name="prep", bufs=6) as prepool,
            tc.tile_pool(name="gatep", bufs=2) as gatepool,
            tc.tile_pool(name="woutp", bufs=3) as wopool,
            tc.tile_pool(name="outp", bufs=4) as opool,
            tc.tile_pool(name="psum", bufs=8, space="PSUM") as pspool,
        ):
            # persistent SBUF state
            wh_sb = cpool.tile([P, NQ, 2, G4], FP8, tag="wh")
            c_sb = cpool.tile([P, NBT, H], F32, tag="c")
            idx_sb = cpool.tile([P, NG], I32, tag="idx")
            # h feature-major: bf16 staging (DMA-transpose dst) and fp8
            # (matmul lhsT), double-buffered by step parity
            hf16 = [hpool.tile([P, KH, BS], BF16, tag=f"hf16_{i}",
                               name=f"hf16_{i}") for i in range(2)]
            hf8 = [hpool.tile([P, KH, BS], FP8, tag=f"hf8_{i}",
                              name=f"hf8_{i}") for i in range(2)]

            nc.sync.dma_start(out=idx_sb[:], in_=idx_d.ap())
            nc.sync.dma_start(out=wh_sb[:], in_=wh_d.ap())

            # all embedding-projection gathers issued upfront; they pipeline
            # on the dynamic DMA queue ahead of the recurrence.
            xgs = []
            for g in range(NG):
                xg = gpool.tile([P, G4], BF16, tag="xg")
                nc.gpsimd.indirect_dma_start(
                    out=xg[:],
                    out_offset=None,
                    in_=xt_d.ap(),
                    in_offset=IndirectOffsetOnAxis(ap=idx_sb[:, g:g + 1], axis=0),
                )
                xgs.append(xg)

            GATES = ((0, "f", AF.Sigmoid), (1, "i", AF.Sigmoid),
                     (2, "g", AF.Tanh), (3, "o", AF.Sigmoid))

            def cell_math(t, bt, hh, figo, xg_or_pss):
                """Emit acts + cell update for one (bt, hh) half-block."""
                cs = c_sb[:, bt, hh * 512:(hh + 1) * 512]
                if t > 0:
                    nc.vector.tensor_mul(out=cs, in0=figo["f"][:], in1=cs)
                    nc.vector.tensor_mul(out=figo["g"][:], in0=figo["i"][:],
                                         in1=figo["g"][:])
                    nc.vector.tensor_add(out=cs, in0=cs, in1=figo["g"][:])
                else:
                    nc.vector.tensor_mul(out=cs, in0=figo["i"][:],
                                         in1=figo["g"][:])
                th = prepool.tile([P, 512], F32, tag="th")
                nc.scalar.activation(th[:], cs, AF.Tanh)
                return th

            def emit_quant(t, bt):
                """h_fm bf16 -> fp8 (x SC_H) for one batch-tile column."""
                wbuf = t % 2
                nc.vector.tensor_scalar(
                    out=hf8[wbuf][:, :, bt * P:(bt + 1) * P],
                    in0=hf16[wbuf][:, :, bt * P:(bt + 1) * P],
                    scalar1=SC_H, scalar2=None, op0=ALU.mult)

            # ---- t = 0: gates come straight from the gathered table ----
            pending_quant = []
            for bt in range(NBT):
                xg = xgs[bt]
                hbm = hbmpool.tile([P, H], BF16, tag="hbm")
                for hh in range(2):
                    figo = {}
                    for gi, nm, fn in GATES:
                        if nm == "f":
                            continue  # c0 = 0: forget gate unused at t=0
                        gt = gatepool.tile([P, 512], F32, tag=nm)
                        nc.scalar.activation(
                            gt[:], xg[:, gi * H + hh * 512: gi * H + hh * 512 + 512],
                            fn)
                        figo[nm] = gt
                    th = cell_math(0, bt, hh, figo, xg)
                    nc.vector.tensor_mul(
                        out=hbm[:, hh * 512:(hh + 1) * 512],
                        in0=figo["o"][:], in1=th[:])
                nc.scalar.dma_start_transpose(
                    hf16[0][:, :, bt * P:(bt + 1) * P], hbm[:])
                pending_quant.append((0, bt))
                if len(pending_quant) > 1:
                    emit_quant(*pending_quant.pop(0))
            while pending_quant:
                emit_quant(*pending_quant.pop(0))

            # ---- steps t = 1..4 ----
            for t in range(1, T):
                rbuf, wbuf = (t + 1) % 2, t % 2
                last = t == T - 1
                for bt in range(NBT):
                    pss = [pspool.tile([P, 512], F32, tag="ps", name="ps")
                           for _ in range(8)]
                    for q in range(NQ):
                        lhsT = hf8[rbuf][:, 2 * q:2 * q + 2, bt * P:(bt + 1) * P]
                        for ch in range(8):
                            nc.tensor.matmul(
                                pss[ch][:],
                                lhsT=lhsT,
                                rhs=wh_sb[:, q, :, ch * 512:(ch + 1) * 512],
                                perf_mode=DR,
                                start=(q == 0),
                                stop=(q == NQ - 1),
                            )
                    xg = xgs[t * NBT + bt]
                    hbm = hbmpool.tile([P, H], BF16, tag="hbm")
                    for hh in range(2):
                        figo = {}
                        for gi, nm, fn in GATES:
                            ch = gi * 2 + hh
                            pre = prepool.tile([P, 512], F32, tag="pre")
                            nc.vector.scalar_tensor_tensor(
                                out=pre[:], in0=pss[ch][:], scalar=DESCALE,
                                in1=xg[:, gi * H + hh * 512: gi * H + hh * 512 + 512],
                                op0=ALU.mult, op1=ALU.add)
                            gt = gatepool.tile([P, 512], F32, tag=nm)
                            nc.scalar.activation(gt[:], pre[:], fn)
                            figo[nm] = gt
                        th = cell_math(t, bt, hh, figo, None)
                        nc.vector.tensor_mul(
                            out=hbm[:, hh * 512:(hh + 1) * 512],
                            in0=figo["o"][:], in1=th[:])
                    nc.scalar.dma_start_transpose(
                        hf16[wbuf][:, :, bt * P:(bt + 1) * P], hbm[:])
                    if not last:
                        pending_quant.append((t, bt))
                        if len(pending_quant) > 1:
                            emit_quant(*pending_quant.pop(0))
                while pending_quant:
                    emit_quant(*pending_quant.pop(0))

            # ---- output projection (h5 = hf16[(T-1) % 2], bf16) ----
            h5 = hf16[(T - 1) % 2]
            QW = KH * VN // 4  # wout tile loaded in 4 quarters for overlap
            for vt in range(VT):
                vn = min(VN, V - vt * VN)
                wo_sb = wopool.tile([P, KH * VN], BF16, tag="wo")
                for qq in range(4):
                    nc.sync.dma_start(out=wo_sb[:, qq * QW:(qq + 1) * QW],
                                      in_=wo_d.ap()[vt][:, qq * QW:(qq + 1) * QW])
                for bt in range(NBT):
                    ps = pspool.tile([P, VN], F32, tag="ps")
                    for k in range(KH):
                        nc.tensor.matmul(
                            ps[:, :vn],
                            lhsT=h5[:, k, bt * P:(bt + 1) * P],
                            rhs=wo_sb[:, k * VN:k * VN + vn],
                            start=(k == 0),
                            stop=(k == KH - 1),
                        )
                    ot = opool.tile([P, VN], F32, tag="ot")
                    nc.vector.tensor_copy(out=ot[:, :vn], in_=ps[:, :vn])
                    # logit writes go out on the ACT HWDGE queue so they
                    # don't contend with the wout reads on the sync queue
                    nc.scalar.dma_start(
                        out=out_d.ap()[bt * P:(bt + 1) * P,
                                       vt * VN:vt * VN + vn],
                        in_=ot[:, :vn])

    nc.compile()
    return nc


def get_nc():
    if "nc" not in _CACHE:
        _CACHE["nc"] = _build()
    return _CACHE["nc"]


def _prep_shared(Emb, WF, WI, WC, WO, bF, bI, bC, bO, Wout):
    Wcat = np.concatenate([np.asarray(WF), np.asarray(WI), np.asarray(WC),
                           np.asarray(WO)], 0).astype(np.float32)  # [4096,1536]
    bcat = np.concatenate([np.asarray(bF), np.asarray(bI), np.asarray(bC),
                           np.asarray(bO)], 0).astype(np.float32)  # [4096]

    # x-path fold: Emb @ Wx.T + b -> bf16 gather table [32000, 4096]
    Emb32 = np.asarray(Emb, dtype=np.float32)
    xt = (Emb32 @ Wcat[:, H:].T + bcat[None, :]).astype(_BF)

    # h-path weights, fp8, DoubleRow pairing: wh[p, q, i, g] = Wh[(2q+i)*128+p, g]
    Wh = Wcat[:, :H].T  # [1024, 4096]
    wh = np.ascontiguousarray(
        (Wh * SC_W).reshape(NQ, 2, P, G4).transpose(2, 0, 1, 3)).astype(_F8)

    Wout = np.asarray(Wout, dtype=np.float32)
    wpad = np.zeros((VPAD, H), np.float32)
    wpad[:V] = Wout
    wo = np.ascontiguousarray(
        wpad.reshape(VT, VN, KH, P).transpose(0, 3, 2, 1).reshape(VT, P, KH * VN)
    ).astype(_BF)  # [63, 128, 4096]
    return xt, wh, wo


def kernel(X, Emb, WF, bF, WI, bI, WC, bC, WO, bO, Wout, bout):
    global LAST_RESULTS
    nc = get_nc()

    xt, wh, wo = _prep_shared(Emb, WF, WI, WC, WO, bF, bI, bC, bO, Wout)
    X = np.asarray(X).astype(np.int32)  # [4096, 5]

    in_maps = []
    for c in range(NCORES):
        xs = X[c * BS:(c + 1) * BS]                       # [512, 5]
        idx = np.ascontiguousarray(
            xs.T.reshape(NG, P).T).astype(np.int32)       # [128, 20] t-major
        in_maps.append({"idx": idx, "xt": xt, "wh": wh, "wo": wo})

    res = run_bass_kernel_spmd(nc, in_maps, core_ids=list(range(NCORES)))
    LAST_RESULTS = res

    out = np.concatenate([res.results[c]["out"] for c in range(NCORES)], 0)
    bout = np.asarray(bout, dtype=np.float32)
    if np.any(bout):
        out = out + bout[None, :]
    return out


# revision 6
# speedup vs baseline: 1.2534x; 1.0532x over previous
"""TextLSTM kernel for 8 Trainium2 NeuronCores.

Data-parallel over batch: each of the 8 cores runs the full model on a
512-row batch shard.

Structure (v4):
  1. Host folds Emb @ Wx.T + b into a [32000, 4096] bf16 table scaled by
     2^21 (= fp8 h-scale * fp8 W-scale), gathered per token via indirect
     DMA (t-major, 8KB rows). No x matmuls, and t=0 needs no matmuls at
     all (h0 = 0).
  2. Gates batch-major in 2-bank PSUM tiles [128b, 1024]: each 512-col
     half accumulates {identity-matmul of the gathered-table chunk (bf16,
     injects the x-contribution already in the 2^21 domain)} + {4 fp8
     DoubleRow matmuls (K=256 each) of h against Wh}. ScalarE activates
     1024-wide straight from the PSUM pair with scale=2^-21, emitting
     bf16 gates; VectorE does the cell math 1024-wide in bf16 (2x DVE
     rate); cell state c is bf16 (verified: rel err 0.0095 < 2e-2).
  3. h transposes to feature-major via SBUF->SBUF DMA-transpose on the
     sync queue (bf16), then one per-batch-tile fp8 quantize (x 2^13) on
     VectorE feeds the next step's DoubleRow lhsT.
  4. Projection: out[512b, 32000v] = h5.T @ WoutT per 512-col vocab tile,
     bf16 weights (fp8 breaks the 2e-2 budget: measured 3.7e-2), fp32
     PSUM shared by batch-tile pairs, bf16 output staged and written on
     the ACT queue (host upcasts to fp32).
  5. PE warmup matmuls (identity) paced through t=0 keep the HAM clock
     gate at 8/8 before the recurrence starts.
"""

import os
import sys

import numpy as np
import ml_dtypes

for _p in ("/opt/trn_rl_repo", "/root/.axon_site/_ro/trn_rl_repo"):
    if os.path.isdir(_p) and _p not in sys.path:
        sys.path.append(_p)

from concourse import bacc, mybir
import concourse.tile as tile
from concourse.bass import IndirectOffsetOnAxis
from concourse.bass_utils import run_bass_kernel_spmd

P = 128
B, T, E, H, V = 4096, 5, 512, 1024, 32000
NCORES = 8
BS = B // NCORES          # 512 batch rows per core
NBT = BS // P             # 4 batch tiles
NG = NBT * T              # 20 gather tiles of 128 tokens
G4 = 4 * H                # 4096 gate pre-activations per token
KH = H // P               # 8 k-tiles over h
NQ = KH // 2              # 4 DoubleRow k-pairs
VN = 512                  # vocab tile width
VT = (V + VN - 1) // VN   # 63 vocab tiles (last one 256 wide)
VPAD = VT * VN            # 32256

SC_H = 8192.0             # h -> fp8 scale (2^13)
SC_W = 256.0              # Wh -> fp8 scale (2^8)
XSCALE = SC_H * SC_W      # table pre-scale (2^21)
DESCALE = 1.0 / XSCALE

F32 = mybir.dt.float32
BF16 = mybir.dt.bfloat16
FP8 = mybir.dt.float8e4
I32 = mybir.dt.int32
AF = mybir.ActivationFunctionType
ALU = mybir.AluOpType
DR = mybir.MatmulPerfMode.DoubleRow

_BF = ml_dtypes.bfloat16
_F8 = ml_dtypes.float8_e4m3fn

_CACHE = {}
LAST_RESULTS = None


def _build():
    nc = bacc.Bacc("TRN2", target_bir_lowering=False, debug=False,
                   num_devices=NCORES)

    idx_d = nc.dram_tensor("idx", [P, NG], I32, kind="ExternalInput")
    id_d = nc.dram_tensor("ident", [P, P], BF16, kind="ExternalInput")
    xt_d = nc.dram_tensor("xt", [V, G4], BF16, kind="ExternalInput")
    wh_d = nc.dram_tensor("wh", [P, NQ, 2, G4], FP8, kind="ExternalInput")
    wo_d = nc.dram_tensor("wo", [VT, P, KH * VN], BF16, kind="ExternalInput")
    out_d = nc.dram_tensor("out", [BS, V], BF16, kind="ExternalOutput")

    with tile.TileContext(nc) as tc:
        with (
            tc.tile_pool(name="const", bufs=1) as cpool,
            tc.tile_pool(name="gather", bufs=6) as gpool,
            tc.tile_pool(name="hstate", bufs=1) as hpool,
            tc.tile_pool(name="hbmp", bufs=3) as hbmpool,
            tc.tile_pool(name="gatep", bufs=2) as gatepool,
            tc.tile_pool(name="thp", bufs=2) as thpool,
            tc.tile_pool(name="woutp", bufs=3) as wopool,
            tc.tile_pool(name="outp", bufs=4) as opool,
            tc.tile_pool(name="psum", bufs=4, space="PSUM") as pspool,
        ):
            # persistent SBUF state
            wh_sb = cpool.tile([P, NQ, 2, G4], FP8, tag="wh")
            c_sb = cpool.tile([P, NBT, H], BF16, tag="c")
            idx_sb = cpool.tile([P, NG], I32, tag="idx")
            ident = cpool.tile([P, P], BF16, tag="ident")
            hf16 = [hpool.tile([P, KH, BS], BF16, tag=f"hf16_{i}",
                               name=f"hf16_{i}") for i in range(2)]
            hf8 = [hpool.tile([P, KH, BS], FP8, tag=f"hf8_{i}",
                              name=f"hf8_{i}") for i in range(2)]

            nc.sync.dma_start(out=idx_sb[:], in_=idx_d.ap())
            nc.sync.dma_start(out=ident[:], in_=id_d.ap())
            nc.sync.dma_start(out=wh_sb[:], in_=wh_d.ap())

            # all table gathers issued upfront; they pipeline on the
            # dynamic DMA queue ahead of the recurrence.
            xgs = []
            for g in range(NG):
                xg = gpool.tile([P, G4], BF16, tag="xg")
                nc.gpsimd.indirect_dma_start(
                    out=xg[:],
                    out_offset=None,
                    in_=xt_d.ap(),
                    in_offset=IndirectOffsetOnAxis(ap=idx_sb[:, g:g + 1], axis=0),
                )
                xgs.append(xg)

            def warm_mms(n, rhs):
                """Dummy matmuls: keep the PE HAM clock-gate open."""
                for _ in range(n):
                    wps = pspool.tile([P, 2 * VN], F32, tag="ps2", name="wps")
                    nc.tensor.matmul(wps[:, :rhs.shape[-1]], lhsT=ident[:],
                                     rhs=rhs, start=True, stop=True)

            warm_mms(16, ident[:])

            GATES = ((0, "f", AF.Sigmoid), (1, "i", AF.Sigmoid),
                     (2, "g", AF.Tanh), (3, "o", AF.Sigmoid))

            def emit_quant(t, bt):
                """h_fm bf16 -> fp8 (x SC_H) for one batch-tile column."""
                wbuf = t % 2
                nc.vector.tensor_scalar(
                    out=hf8[wbuf][:, :, bt * P:(bt + 1) * P],
                    in0=hf16[wbuf][:, :, bt * P:(bt + 1) * P],
                    scalar1=SC_H, scalar2=None, op0=ALU.mult)

            # ---- t = 0: gates come straight from the gathered table ----
            for bt in range(NBT):
                xg = xgs[bt]
                hbm = hbmpool.tile([P, H], BF16, tag="hbm")
                figo = {}
                for gi, nm, fn in GATES:
                    if nm == "f":
                        continue  # c0 = 0: forget gate unused at t=0
                    gt = gatepool.tile([P, H], BF16, tag=nm)
                    nc.scalar.activation(gt[:], xg[:, gi * H:(gi + 1) * H],
                                         fn, scale=DESCALE)
                    figo[nm] = gt
                cs = c_sb[:, bt, :]
                nc.vector.tensor_mul(out=cs, in0=figo["i"][:], in1=figo["g"][:])
                th = thpool.tile([P, H], BF16, tag="th")
                nc.scalar.activation(th[:], cs, AF.Tanh)
                nc.vector.tensor_mul(out=hbm[:], in0=figo["o"][:], in1=th[:])
                nc.sync.dma_start_transpose(
                    hf16[0][:, :, bt * P:(bt + 1) * P], hbm[:])
                emit_quant(0, bt)
                warm_mms(8, hbm[:, 0:VN])  # paced PE keep-warm during t0

            # ---- steps t = 1..4 ----
            for t in range(1, T):
                rbuf, wbuf = (t + 1) % 2, t % 2
                last = t == T - 1
                pending_quant = []
                for bt in range(NBT):
                    xg = xgs[t * NBT + bt]
                    pss = [pspool.tile([P, 2 * VN], F32, tag="ps2", name="ps2")
                           for _ in range(4)]
                    # x-contribution injected via identity matmul (opens
                    # each accumulation half-group), already x2^21 on host
                    for gi in range(4):
                        for hh in range(2):
                            nc.tensor.matmul(
                                pss[gi][:, hh * VN:(hh + 1) * VN],
                                lhsT=ident[:],
                                rhs=xg[:, gi * H + hh * 512: gi * H + hh * 512 + 512],
                                start=True, stop=False)
                    for q in range(NQ):
                        lhsT = hf8[rbuf][:, 2 * q:2 * q + 2, bt * P:(bt + 1) * P]
                        for gi in range(4):
                            for hh in range(2):
                                nc.tensor.matmul(
                                    pss[gi][:, hh * VN:(hh + 1) * VN],
                                    lhsT=lhsT,
                                    rhs=wh_sb[:, q, :,
                                              gi * H + hh * 512: gi * H + hh * 512 + 512],
                                    perf_mode=DR,
                                    start=False,
                                    stop=(q == NQ - 1),
                                )
                    hbm = hbmpool.tile([P, H], BF16, tag="hbm")
                    figo = {}
                    for gi, nm, fn in GATES:
                        gt = gatepool.tile([P, H], BF16, tag=nm)
                        nc.scalar.activation(gt[:], pss[gi][:], fn,
                                             scale=DESCALE)
                        figo[nm] = gt
                    cs = c_sb[:, bt, :]
                    nc.vector.tensor_mul(out=cs, in0=figo["f"][:], in1=cs)
                    nc.vector.tensor_mul(out=figo["g"][:], in0=figo["i"][:],
                                         in1=figo["g"][:])
                    nc.vector.tensor_add(out=cs, in0=cs, in1=figo["g"][:])
                    th = thpool.tile([P, H], BF16, tag="th")
                    nc.scalar.activation(th[:], cs, AF.Tanh)
                    nc.vector.tensor_mul(out=hbm[:], in0=figo["o"][:],
                                         in1=th[:])
                    nc.sync.dma_start_transpose(
                        hf16[wbuf][:, :, bt * P:(bt + 1) * P], hbm[:])
                    if not last:
                        pending_quant.append((t, bt))
                        if len(pending_quant) > 1:
                            emit_quant(*pending_quant.pop(0))
                while pending_quant:
                    emit_quant(*pending_quant.pop(0))

            # ---- output projection (h5 = hf16[(T-1) % 2], bf16) ----
            h5 = hf16[(T - 1) % 2]
            QW = KH * VN // 4  # wout tile loaded in 4 quarters for overlap
            for vt in range(VT):
                vn = min(VN, V - vt * VN)
                wo_sb = wopool.tile([P, KH * VN], BF16, tag="wo")
                for qq in range(4):
                    nc.sync.dma_start(out=wo_sb[:, qq * QW:(qq + 1) * QW],
                                      in_=wo_d.ap()[vt][:, qq * QW:(qq + 1) * QW])
                for bp in range(NBT // 2):  # batch-tile pairs share a PSUM pair
                    ps = pspool.tile([P, 2 * VN], F32, tag="ps2", name="psp")
                    for bi in range(2):
                        bt = bp * 2 + bi
                        for k in range(KH):
                            nc.tensor.matmul(
                                ps[:, bi * VN:bi * VN + vn],
                                lhsT=h5[:, k, bt * P:(bt + 1) * P],
                                rhs=wo_sb[:, k * VN:k * VN + vn],
                                start=(k == 0),
                                stop=(k == KH - 1),
                            )
                    ot = opool.tile([P, 2 * VN], BF16, tag="ot")
                    nc.vector.tensor_copy(out=ot[:], in_=ps[:])
                    # logit writes go out on the ACT HWDGE queue so they
                    # don't contend with the wout reads on the sync queue
                    for bi in range(2):
                        bt = bp * 2 + bi
                        nc.scalar.dma_start(
                            out=out_d.ap()[bt * P:(bt + 1) * P,
                                           vt * VN:vt * VN + vn],
                            in_=ot[:, bi * VN:bi * VN + vn])

    nc.compile()
    return nc


def get_nc():
    if "nc" not in _CACHE:
        _CACHE["nc"] = _build()
    return _CACHE["nc"]


def _prep_shared(Emb, WF, WI, WC, WO, bF, bI, bC, bO, Wout):
    Wcat = np.concatenate([np.asarray(WF), np.asarray(WI), np.asarray(WC),
                           np.asarray(WO)], 0).astype(np.float32)  # [4096,1536]
    bcat = np.concatenate([np.asarray(bF), np.asarray(bI), np.asarray(bC),
                           np.asarray(bO)], 0).astype(np.float32)  # [4096]

    # x-path fold: (Emb @ Wx.T + b) * 2^21 -> bf16 gather table [32000, 4096]
    Emb32 = np.asarray(Emb, dtype=np.float32)
    xt = ((Emb32 @ Wcat[:, H:].T + bcat[None, :]) * XSCALE).astype(_BF)

    # h-path weights, fp8, DoubleRow pairing: wh[p, q, i, g] = Wh[(2q+i)*128+p, g]
    Wh = Wcat[:, :H].T  # [1024, 4096]
    wh = np.ascontiguousarray(
        (Wh * SC_W).reshape(NQ, 2, P, G4).transpose(2, 0, 1, 3)).astype(_F8)

    Wout = np.asarray(Wout, dtype=np.float32)
    wpad = np.zeros((VPAD, H), np.float32)
    wpad[:V] = Wout
    wo = np.ascontiguousarray(
        wpad.reshape(VT, VN, KH, P).transpose(0, 3, 2, 1).reshape(VT, P, KH * VN)
    ).astype(_BF)  # [63, 128, 4096]
    return xt, wh, wo


def kernel(X, Emb, WF, bF, WI, bI, WC, bC, WO, bO, Wout, bout):
    global LAST_RESULTS
    nc = get_nc()

    xt, wh, wo = _prep_shared(Emb, WF, WI, WC, WO, bF, bI, bC, bO, Wout)
    X = np.asarray(X).astype(np.int32)  # [4096, 5]
    identity = np.eye(P, dtype=_BF)

    in_maps = []
    for c in range(NCORES):
        xs = X[c * BS:(c + 1) * BS]                       # [512, 5]
        idx = np.ascontiguousarray(
            xs.T.reshape(NG, P).T).astype(np.int32)       # [128, 20] t-major
        in_maps.append({"idx": idx, "ident": identity, "xt": xt, "wh": wh,
                        "wo": wo})

    res = run_bass_kernel_spmd(nc, in_maps, core_ids=list(range(NCORES)))
    LAST_RESULTS = res

    out = np.concatenate(
        [res.results[c]["out"].astype(np.float32) for c in range(NCORES)], 0)
    bout = np.asarray(bout, dtype=np.float32)
    if np.any(bout):
        out = out + bout[None, :]
    return out
